# revision 19
# baseline (speedup 1.0000x reference)
"""Trainium2 Bass kernel: NeRF fine-sampling (inverse-CDF sample + merge-sort).

Contract: kernel(**inputs) takes the FULL inputs
    dists         [262144, 64]  f32  (per-ray sorted distances)
    weights       [262144, 63]  f32
    uniform_rands [262144, 128] f32
    samples_fine  scalar (= 128)
and returns the FULL output [262144, 192] f32, equal to
    sort(concat(inverse_cdf_samples, dists), axis=-1).

Strategy: pure data-parallel over rays; 8 NeuronCores each take 32768 rays.
Ray r = p*8+g maps to partition p, segment g, so every DMA moves one
contiguous 2-6KB run per partition (128 descriptors/DMA).  The per-group
pipeline runs inside a For_i hardware loop with register-offset (ds) DRAM
slices: the sequencers replay a small resident body instead of fetching a
fully unrolled instruction stream, which dominates cost on this runtime.

Algorithm (per ray; 128 rays per 128-partition tile, G=8 tiles per group):
  The piecewise-linear CDF F passes through points (dists_j, cdf_j), so the
  output is exactly F^{-1}(sort(u ++ cdf)): merge the 64 cdf breakpoints
  with the 128 u values, then evaluate the piecewise-linear F^{-1} at every
  sorted key.  Breakpoints get their fp32 LSB set, u values LSB cleared, so
  after the value sort a breakpoint is identified by its LSB.  u is sorted
  ascending by a 28-stage bitonic mergesort network (fat ops across G
  tiles); the (already sorted)
  breakpoint run [denormal cdf_0, cdf_1..63, +BIG pad] is merged in with an
  8-stage bitonic merge over each 256 window (windows of G tiles processed
  flat).  GPSIMD local_scatter places each bin's (d_j, slope_j) halfword
  pair at its breakpoint's sorted position; masked forward-fill scans
  (state = (1-tag)*state + data) recover (cdf0, d0, slope) at every
  position, and out = d0 + (v - cdf0) * slope.  Breakpoint entries evaluate
  to exactly d_j, reproducing the coarse dists in the merged result.
"""

from contextlib import ExitStack

import numpy as np

import concourse.bass as bass
import concourse.tile as tile
from concourse import bacc, mybir

F32 = mybir.dt.float32
I32 = mybir.dt.int32
I16 = mybir.dt.int16
U16 = mybir.dt.uint16
Alu = mybir.AluOpType
Act = mybir.ActivationFunctionType

P = 128
SC = 64
NW = SC - 1      # 63
SF = 128
NV = SF + NW     # 191
OUT = SF + SC    # 192
W = 256          # per-segment merge window
G = 8            # ray-tiles per group

NEG = -1e30
BIG = 1e30


def _r3(ap, inner):
    return ap.rearrange("p (g w) -> p g w", w=inner)


def emit_group(nc, pools, dists_ap, weights_ap, u_ap, out_ap, consts):
    """Process G ray-tiles (G*128 consecutive rays)."""
    io_pool, front_pool, big_pool, sc_pool = pools
    iotaE = consts["iotaE"]      # i16 [P, G*OUT]: e (0..191) per segment
    bias01 = consts["bias01"]    # f32 [P, 1] = 0.01

    # ---- load (one DMA per tensor per group; row (p*8+g) -> [p, seg g]) ----
    dQ = io_pool.tile([P, G * SC], F32, tag="dQ")
    nc.sync.dma_start(dQ[:], dists_ap.rearrange("(p g) c -> p (g c)", g=G))
    wQ = io_pool.tile([P, G * NW], F32, tag="wQ")
    nc.sync.dma_start(wQ[:], weights_ap.rearrange("(p g) c -> p (g c)", g=G))
    V = front_pool.tile([P, G * SF], F32, tag="V")
    nc.sync.dma_start(V[:], u_ap.rearrange("(p g) c -> p (g c)", g=G))

    # XT window per segment g (cols g*W..g*W+255):
    #   [0:128]   u sorted descending
    #   [128]     tagged zero (denormal 0x1 = cdf_0 breakpoint)
    #   [129:192] tagged cdf_1..63 ascending
    #   [192:256] +BIG pad
    # -> a bitonic valley; after the merge, cols [0:192] of each window are
    #    the 192 sorted keys (breakpoints LSB=1).
    XT = front_pool.tile([P, G * W], F32, tag="XT")
    XT_i = XT[:].bitcast(I32)
    XT3 = _r3(XT[:], W)
    XT3_i = _r3(XT_i, W)

    # ---- cdf ----
    w1 = front_pool.tile([P, G * NW], F32, tag="w1")
    nc.scalar.activation(w1[:], wQ[:], Act.Identity, bias=bias01[:])
    cw = front_pool.tile([P, G * NW], F32, tag="cw")
    nc.vector.tensor_tensor_scan(cw[:], consts["wmask"][:], w1[:], 0.0,
                                 Alu.mult, Alu.add)
    rec = front_pool.tile([P, G], F32, tag="rec")
    nc.vector.reciprocal(rec[:], cw[:, NW - 1::NW])
    for g in range(G):
        nc.scalar.activation(XT[:, g * W + 129:g * W + 192],
                             cw[:, g * NW:(g + 1) * NW], Act.Copy,
                             scale=rec[:, g:g + 1])

    # ---- LSB tagging + pads ----
    nc.vector.tensor_scalar(out=V[:].bitcast(I32), in0=V[:].bitcast(I32),
                            scalar1=-2, scalar2=None, op0=Alu.bitwise_and)
    nc.vector.tensor_scalar(out=XT3_i[:, :, 129:192], in0=XT3_i[:, :, 129:192],
                            scalar1=1, scalar2=None, op0=Alu.bitwise_or)
    nc.vector.memset(XT3_i[:, :, 128:129], -1082130431)  # -1.0|LSB
    nc.vector.memset(XT3[:, :, 192:256], BIG)

    # ---- per-bin slope (before the merge clobbers XT's cdf cols) ----
    dQ3 = _r3(dQ[:], SC)
    ddiff = sc_pool.tile([P, G * SC], F32, tag="ddiff")
    ddiff3 = _r3(ddiff[:], SC)
    nc.vector.tensor_tensor(ddiff3[:, :, 0:NW], dQ3[:, :, 1:SC],
                            dQ3[:, :, 0:NW], Alu.subtract)
    nc.vector.memset(ddiff3[:, :, NW:SC], 0.0)
    cdiff = sc_pool.tile([P, G * SC], F32, tag="cdiff")
    cdiff3 = _r3(cdiff[:], SC)
    nc.vector.tensor_copy(cdiff3[:, :, 0:1], XT3[:, :, 129:130])
    nc.vector.tensor_tensor(cdiff3[:, :, 1:NW], XT3[:, :, 130:192],
                            XT3[:, :, 129:191], Alu.subtract)
    nc.vector.memset(cdiff3[:, :, NW:SC], 1.0)
    rcd = sc_pool.tile([P, G * SC], F32, tag="rcd")
    nc.vector.reciprocal(rcd[:], cdiff[:])
    slope = sc_pool.tile([P, G * SC], F32, tag="slope")
    nc.vector.tensor_tensor(slope[:], ddiff[:], rcd[:], Alu.mult)

    # ---- sort u ascending in V via bitonic mergesort (all-ascending runs;
    #      each level's first stage reads the first half-run reversed) ----
    V2b = front_pool.tile([P, G * SF], F32, tag="V2b")
    bufsv = [V, V2b]
    cur = 0
    for lev in range(1, 8):          # k = 2, 4, ..., 128
        k = 1 << lev
        h = k // 2
        src = bufsv[cur][:].rearrange("p (c b) -> p c b", b=k)
        dst = bufsv[1 - cur][:].rearrange("p (c b) -> p c b", b=k)
        lo_rev = src[:, :, h - 1::-1]
        hi = src[:, :, h:k]
        nc.vector.tensor_tensor(dst[:, :, 0:h], lo_rev, hi, Alu.min)
        nc.vector.tensor_tensor(dst[:, :, h:k], lo_rev, hi, Alu.max)
        cur = 1 - cur
        s = k // 4
        while s >= 1:
            src = bufsv[cur][:].rearrange("p (c b) -> p c b", b=2 * s)
            dst = bufsv[1 - cur][:].rearrange("p (c b) -> p c b", b=2 * s)
            nc.vector.tensor_tensor(dst[:, :, 0:s], src[:, :, 0:s],
                                    src[:, :, s:2 * s], Alu.min)
            nc.vector.tensor_tensor(dst[:, :, s:2 * s], src[:, :, 0:s],
                                    src[:, :, s:2 * s], Alu.max)
            cur = 1 - cur
            s //= 2
    assert cur == 0  # 28 stages -> sorted ascending back in V
    Vsrt3 = V[:].rearrange("p (g c) -> p g c", c=SF)

    # ---- bitonic merge of [u-asc (read reversed) | denorm+cdf+BIG] ----
    # Stage 1 reads u from V (reversed) and the ascending breakpoint run from
    # XT's window second half, writing Y; stages 2..8 ping-pong Y<->XT and
    # land back in XT.
    Y = front_pool.tile([P, G * W], F32, tag="Y")
    Y3 = _r3(Y[:], W)
    nc.vector.tensor_tensor(Y3[:, :, 0:W // 2], Vsrt3[:, :, SF - 1::-1],
                            XT3[:, :, W // 2:W], Alu.min)
    nc.vector.tensor_tensor(Y3[:, :, W // 2:W], Vsrt3[:, :, SF - 1::-1],
                            XT3[:, :, W // 2:W], Alu.max)
    bufs = [Y, XT]
    s = W // 4
    idx = 0
    while s >= 1:
        src = bufs[idx % 2][:].rearrange("p (a b) -> p a b", b=2 * s)
        dst = bufs[(idx + 1) % 2][:].rearrange("p (a b) -> p a b", b=2 * s)
        nc.vector.tensor_tensor(dst[:, :, 0:s], src[:, :, 0:s],
                                src[:, :, s:2 * s], Alu.min)
        nc.vector.tensor_tensor(dst[:, :, s:2 * s], src[:, :, 0:s],
                                src[:, :, s:2 * s], Alu.max)
        s //= 2
        idx += 1
    assert idx % 2 == 1  # 7 stages after stage 1 -> result back in XT
    Vs3 = XT3[:, :, 0:OUT]
    Vs3_i = XT3_i[:, :, 0:OUT]

    # ---- tags (Pool/Act; DVE stays on the sort/merge/scan path) ----
    tagi = big_pool.tile([P, G * OUT], I32, tag="tagi")
    nc.vector.tensor_scalar(out=_r3(tagi[:], OUT), in0=Vs3_i, scalar1=1,
                            scalar2=None, op0=Alu.bitwise_and)
    tag = big_pool.tile([P, G * OUT], F32, tag="tag")
    nc.scalar.activation(tag[:], tagi[:], Act.Copy)
    omt = big_pool.tile([P, G * OUT], F32, tag="omt")
    nc.scalar.activation(omt[:], tagi[:], Act.Copy, scale=-1.0, bias=1.0)

    # ---- breakpoint positions ----
    C = big_pool.tile([P, G * OUT], F32, tag="C")
    nc.vector.tensor_tensor_scan(C[:], tag[:], tag[:], 0.0, Alu.add, Alu.bypass)
    # flat cumsum carries 64g across segments: idxf = C*tag-1 = 64g+j / -1
    idxf = big_pool.tile([P, G * OUT], F32, tag="idxf")
    nc.vector.tensor_tensor(idxf[:], C[:], tag[:], Alu.mult)
    idx16 = big_pool.tile([P, G * OUT], I16, tag="idx16")
    nc.scalar.activation(idx16[:], idxf[:], Act.Copy, scale=1.0, bias=-1.0)
    posTab = sc_pool.tile([P, G * SC], I16, tag="posTab")
    nc.gpsimd.local_scatter(posTab[:], iotaE[:], idx16[:],
                            channels=P, num_elems=G * SC, num_idxs=G * OUT)

    # ---- halfword scatter indices: posTab(i16) -> dIdx16(i16) directly ----
    H = G * SF // 2          # 512 idx per half
    HP = G * SC // 2         # 256 posTab entries per half
    dIdx16 = sc_pool.tile([P, G * SF], I16, tag="dIdx16")
    nc.scalar.activation(dIdx16[:, 0:H:2], posTab[:, 0:HP], Act.Copy,
                         scale=2.0, bias=0.0)
    nc.scalar.activation(dIdx16[:, 1:H:2], posTab[:, 0:HP], Act.Copy,
                         scale=2.0, bias=1.0)
    nc.scalar.activation(dIdx16[:, H::2], posTab[:, HP:], Act.Copy,
                         scale=2.0, bias=-float(2 * OUT * G // 2))
    nc.scalar.activation(dIdx16[:, H + 1::2], posTab[:, HP:], Act.Copy,
                         scale=2.0, bias=-float(2 * OUT * G // 2 - 1))

    # ---- scatter (d_j, slope_j) halfword pairs ----
    HT = G * OUT             # halfwords per half-table (1536)
    dTab16 = big_pool.tile([P, G * 2 * OUT], U16, tag="dTab16")
    sTab16 = big_pool.tile([P, G * 2 * OUT], U16, tag="sTab16")
    dQhw = dQ[:].bitcast(U16)
    slhw = slope[:].bitcast(U16)
    for hh in range(2):
        isl = slice(hh * H, (hh + 1) * H)
        tsl = slice(hh * HT, (hh + 1) * HT)
        nc.gpsimd.local_scatter(dTab16[:, tsl], dQhw[:, isl], dIdx16[:, isl],
                                channels=P, num_elems=HT, num_idxs=H)
        nc.gpsimd.local_scatter(sTab16[:, tsl], slhw[:, isl], dIdx16[:, isl],
                                channels=P, num_elems=HT, num_idxs=H)
    dTab = dTab16[:].bitcast(F32)
    sTab = sTab16[:].bitcast(F32)

    # ---- masked ffills ----
    cdfAt = big_pool.tile([P, G * OUT], F32, tag="cdfAt")
    nc.vector.scalar_tensor_tensor(_r3(cdfAt[:], OUT), Vs3, 0.0,
                                   _r3(tag[:], OUT), Alu.max, Alu.mult)
    C0 = big_pool.tile([P, G * OUT], F32, tag="C0")
    D0 = big_pool.tile([P, G * OUT], F32, tag="D0")
    SLf = big_pool.tile([P, G * OUT], F32, tag="SLf")
    nc.vector.tensor_tensor_scan(C0[:], omt[:], cdfAt[:], 0.0, Alu.mult, Alu.add)
    nc.vector.tensor_tensor_scan(D0[:], omt[:], dTab, dQ[:, 0:1], Alu.mult,
                                 Alu.add)
    nc.vector.tensor_tensor_scan(SLf[:], omt[:], sTab, 0.0, Alu.mult, Alu.add)

    # ---- out = d0 + (v - cdf0) * slope ----
    tnum = big_pool.tile([P, G * OUT], F32, tag="tnum")
    nc.vector.tensor_tensor(_r3(tnum[:], OUT), Vs3, _r3(C0[:], OUT),
                            Alu.subtract)
    outT = big_pool.tile([P, G * OUT], F32, tag="outT")
    nc.vector.scalar_tensor_tensor(outT[:], tnum[:], 0.0, SLf[:], Alu.max,
                                   Alu.mult)
    nc.vector.tensor_tensor(outT[:], outT[:], D0[:], Alu.add)

    nc.sync.dma_start(out_ap.rearrange("(p g) c -> p (g c)", g=G),
                      outT[:])


def build_body(tc, ctx, nc, dists_ap, weights_ap, u_ap, out_ap, n_tiles,
               repeat=1):
    assert n_tiles % G == 0
    io_pool = ctx.enter_context(tc.tile_pool(name="io", bufs=3))
    front_pool = ctx.enter_context(tc.tile_pool(name="front", bufs=3))
    big_pool = ctx.enter_context(tc.tile_pool(name="big", bufs=1))
    sc_pool = ctx.enter_context(tc.tile_pool(name="sc", bufs=1))
    const_pool = ctx.enter_context(tc.tile_pool(name="const", bufs=1))

    iotaE = const_pool.tile([P, G * OUT], I16)
    nc.gpsimd.iota(iotaE[:], pattern=[[OUT, G], [1, OUT]], base=0,
                   channel_multiplier=0)
    wmaski = const_pool.tile([P, G * NW], I16)
    nc.gpsimd.iota(wmaski[:], pattern=[[0, G], [1, NW]], base=0,
                   channel_multiplier=0)
    wmask = const_pool.tile([P, G * NW], F32)
    # 0 -> 0 (segment start), >0 -> 1: min(iota, 1)
    nc.scalar.activation(wmask[:], wmaski[:], Act.Copy)
    nc.vector.tensor_scalar(out=wmask[:], in0=wmask[:], scalar1=1.0,
                            scalar2=None, op0=Alu.min)
    bias01 = const_pool.tile([P, 1], F32)
    nc.vector.memset(bias01[:], 0.01)
    one = const_pool.tile([P, 1], F32)
    nc.vector.memset(one[:], 1.0)

    consts = {"iotaE": iotaE, "bias01": bias01, "wmask": wmask, "one": one}
    pools = (io_pool, front_pool, big_pool, sc_pool)

    # Hardware loop over ray groups (register-offset DRAM slices): the
    # sequencers replay a single resident loop body, so the per-instruction
    # stream-fetch overhead is paid once, not once per group.
    RG = G * P  # rays per group

    def one_group(row0):
        emit_group(nc, pools, dists_ap[bass.ds(row0, RG)],
                   weights_ap[bass.ds(row0, RG)], u_ap[bass.ds(row0, RG)],
                   out_ap[bass.ds(row0, RG)], consts)

    n_rays = n_tiles * P
    assert n_rays % (2 * RG) == 0
    # Two groups per iteration: group B's DMA + Act front-end overlaps
    # group A's DVE sort/merge, and per-iteration loop overhead halves.
    if repeat == 1:
        with tc.For_i(0, n_rays, 2 * RG) as row0:
            one_group(row0)
            one_group(row0 + RG)
    else:
        with tc.For_i(0, repeat):
            with tc.For_i(0, n_rays, 2 * RG) as row0:
                one_group(row0)
                one_group(row0 + RG)


def build_kernel(n_rays, repeat=1):
    assert n_rays % (P * G) == 0
    nc = bacc.Bacc("TRN2", target_bir_lowering=False, debug=False)
    dists = nc.dram_tensor("dists", [n_rays, SC], F32, kind="ExternalInput").ap()
    weights = nc.dram_tensor("weights", [n_rays, NW], F32,
                             kind="ExternalInput").ap()
    u = nc.dram_tensor("u", [n_rays, SF], F32, kind="ExternalInput").ap()
    out = nc.dram_tensor("out", [n_rays, OUT], F32, kind="ExternalOutput").ap()
    with tile.TileContext(nc) as tc:
        with ExitStack() as ctx:
            build_body(tc, ctx, nc, dists, weights, u, out, n_rays // P,
                       repeat=repeat)
    nc.compile()
    return nc


N_CORES = 8
B_FULL = 262144
R_CORE = B_FULL // N_CORES   # 32768 rays per core

_NC_CACHE = {}


def _get_nc(n_rays, repeat=1):
    key = (n_rays, repeat)
    if key not in _NC_CACHE:
        _NC_CACHE[key] = build_kernel(n_rays, repeat)
    return _NC_CACHE[key]


def kernel(dists, weights, uniform_rands, samples_fine):
    from concourse.bass_utils import run_bass_kernel_spmd
    dists = np.ascontiguousarray(np.asarray(dists, dtype=np.float32))
    weights = np.ascontiguousarray(np.asarray(weights, dtype=np.float32))
    u = np.ascontiguousarray(np.asarray(uniform_rands, dtype=np.float32))
    assert int(samples_fine) == SF
    B = dists.shape[0]
    assert B == B_FULL and dists.shape[1] == SC and weights.shape[1] == NW \
        and u.shape[1] == SF

    nc = _get_nc(R_CORE)
    in_maps = []
    for c in range(N_CORES):
        r0, r1 = c * R_CORE, (c + 1) * R_CORE
        in_maps.append({"dists": dists[r0:r1], "weights": weights[r0:r1],
                        "u": u[r0:r1]})
    res = run_bass_kernel_spmd(nc, in_maps, list(range(N_CORES)))
    return np.concatenate([res.results[c]["out"] for c in range(N_CORES)],
                          axis=0)

